# revision 1
# baseline (speedup 1.0000x reference)
"""Trainium2 Bass kernel for ContinuousConv1DSim (gnn_message_passing).

Reformulation (validated vs reference in fp32 numpy, rel err ~4e-5):
  G = F * npm (per-l mask), H = G * t
  MM1  (PE): psw[c2, l] = sum_j GH[j, c2] * Band[j, l]   -- causal 8-wide window
             sums over l, output TRANSPOSED (channels on partitions), with a
             second accumulating matmul adding the previous tile's halo rows.
  MM2a (PE): psp[l, 0:64]  = A_e   (window(G) @ W^T)
             psp[l, 64:128]= D_raw (window(H) @ W^T - window(G) @ bias)
  MM2b (PE): pssp[l, s*64+o] = u[s] * A_e[l, o]          -- s-expansion on PE
  sim_m   = (A_m * t - D_m) with A_m/D_m = npm * psp     (ACT copy w/ scale)
  obuf_sim= pssp * udt + sim_m (broadcast over s)        -- one DVE STT
  real[l] = npm[l] * (t[l] * A_m[l-1] - D_m[l-1])        -- partition-shifted STT
Output rows per l: [real, sim + u_s * udt * A] for s=0..7, last row real[L-1].

Pure data parallel: batch 32 -> 8 cores x 4. All params replicated.
"""

import numpy as np

B, L, C, O, S = 32, 2048, 64, 64, 8
NCORES = 8
BPC = B // NCORES          # 4 batches per core
NT = L // 128              # 16 l-tiles per batch
ROWS = (L - 1) * (S + 1) + 1  # 18424
F32 = None  # set after mybir import


def _consts(W, bias, u):
    n = np.arange(128)
    bandc = ((n[:, None] >= n[None, :] - 7) & (n[:, None] <= n[None, :])).astype(np.float32)
    bandp = (n[:, None] >= n[None, :] + 121).astype(np.float32)
    prba = np.zeros((128, 128), np.float32)
    prba[0:64, 0:64] = W.T           # A_e from U
    prba[0:64, 64:128] = -bias       # -F_e into D_raw
    prba[64:128, 64:128] = W.T       # TA_e into D_raw
    prbb = np.zeros((128, 512), np.float32)
    for s in range(S):
        prbb[0:64, s * 64:(s + 1) * 64] = u[s] * W.T
    return bandc, bandp, prba, prbb


def _build_nc():
    import concourse.bass as bass
    import concourse.bacc as bacc
    import concourse.mybir as mybir
    import concourse.tile as tile

    f32 = mybir.dt.float32
    Copy = mybir.ActivationFunctionType.Copy
    mult = mybir.AluOpType.mult
    sub = mybir.AluOpType.subtract
    add = mybir.AluOpType.add

    nc = bacc.Bacc("TRN2", target_bir_lowering=False, debug=False,
                   num_devices=NCORES)

    FD = nc.dram_tensor("f", [BPC, L, C], f32, kind="ExternalInput").ap()
    TSD = nc.dram_tensor("ts", [BPC, L + 128], f32, kind="ExternalInput").ap()
    UDD = nc.dram_tensor("ud", [BPC, L], f32, kind="ExternalInput").ap()
    NPD = nc.dram_tensor("np", [BPC, L + 128], f32, kind="ExternalInput").ap()
    BCD = nc.dram_tensor("bandc", [128, 128], f32, kind="ExternalInput").ap()
    BPD = nc.dram_tensor("bandp", [128, 128], f32, kind="ExternalInput").ap()
    PAD = nc.dram_tensor("prba", [128, 128], f32, kind="ExternalInput").ap()
    PBD = nc.dram_tensor("prbb", [128, 512], f32, kind="ExternalInput").ap()
    OUTD = nc.dram_tensor("out", [BPC, ROWS, O], f32, kind="ExternalOutput").ap()

    with tile.TileContext(nc) as tc:
        with (
            tc.tile_pool(name="const", bufs=1) as cpool,
            tc.tile_pool(name="scal", bufs=2) as spool,
            tc.tile_pool(name="feat", bufs=3) as fpool,
            tc.tile_pool(name="gh", bufs=3) as ghpool,
            tc.tile_pool(name="sbw", bufs=3) as sbwpool,
            tc.tile_pool(name="pp", bufs=3) as pppool,
            tc.tile_pool(name="simm", bufs=3) as simpool,
            tc.tile_pool(name="ob", bufs=3) as obpool,
            tc.tile_pool(name="ro", bufs=3) as ropool,
            tc.tile_pool(name="psw", bufs=3, space=bass.MemorySpace.PSUM) as pwpool,
            tc.tile_pool(name="psp", bufs=2, space=bass.MemorySpace.PSUM) as papool,
            tc.tile_pool(name="pssp", bufs=2, space=bass.MemorySpace.PSUM) as pbpool,
        ):
            bandc_t = cpool.tile([128, 128], f32, tag="bandc")
            bandp_t = cpool.tile([128, 128], f32, tag="bandp")
            prba_t = cpool.tile([128, 128], f32, tag="prba")
            prbb_t = cpool.tile([128, 512], f32, tag="prbb")
            zrow = cpool.tile([1, 64], f32, tag="zrow")
            nc.sync.dma_start(bandc_t[:], BCD)
            nc.sync.dma_start(bandp_t[:], BPD)
            nc.sync.dma_start(prba_t[:], PAD)
            nc.sync.dma_start(prbb_t[:], PBD)
            nc.gpsimd.memset(zrow[:], 0.0)

            for b in range(BPC):
                tst = spool.tile([128, NT], f32, tag="tst")
                tsh = spool.tile([128, NT], f32, tag="tsh")
                udt = spool.tile([128, NT], f32, tag="udt")
                npt = spool.tile([128, NT], f32, tag="npt")
                nsh = spool.tile([128, NT], f32, tag="nsh")
                nc.sync.dma_start(tst[:], TSD[b, 0:L].rearrange("(n p) -> p n", p=128))
                nc.sync.dma_start(tsh[:], TSD[b, 1:L + 1].rearrange("(n p) -> p n", p=128))
                nc.sync.dma_start(udt[:], UDD[b].rearrange("(n p) -> p n", p=128))
                nc.sync.dma_start(npt[:], NPD[b, 0:L].rearrange("(n p) -> p n", p=128))
                nc.sync.dma_start(nsh[:], NPD[b, 1:L + 1].rearrange("(n p) -> p n", p=128))
                # real row for l=0 is identically zero
                nc.sync.dma_start(OUTD[b, 0:1, :], zrow[:])

                psw_next = None
                for n in range(NT):
                    ftile = fpool.tile([128, C], f32, tag="f")
                    nc.sync.dma_start(ftile[:], FD[b, n * 128:(n + 1) * 128, :])
                    gh = ghpool.tile([128, 128], f32, tag="gh")
                    nc.scalar.activation(gh[:, 0:64], ftile[:], Copy,
                                         scale=npt[:, n:n + 1])
                    nc.vector.tensor_scalar_mul(gh[:, 64:128], gh[:, 0:64],
                                                tst[:, n:n + 1])
                    # MM1: windowed sums, transposed output
                    if n == 0:
                        psw_cur = pwpool.tile([128, 128], f32, tag="psw")
                        nc.tensor.matmul(psw_cur[:], gh[:], bandc_t[:],
                                         start=True, stop=True)
                    else:
                        psw_cur = psw_next
                        nc.tensor.matmul(psw_cur[:], gh[:], bandc_t[:],
                                         start=False, stop=True)
                    if n < NT - 1:
                        psw_next = pwpool.tile([128, 128], f32, tag="psw")
                        nc.tensor.matmul(psw_next[:], gh[:], bandp_t[:],
                                         start=True, stop=False)
                    sbw = sbwpool.tile([128, 128], f32, tag="sbw")
                    nc.scalar.copy(sbw[:], psw_cur[:])
                    # MM2: project windowed features
                    psp = papool.tile([128, 128], f32, tag="psp")
                    nc.tensor.matmul(psp[:], sbw[:], prba_t[:], start=True, stop=True)
                    pssp = pbpool.tile([128, 512], f32, tag="pssp")
                    nc.tensor.matmul(pssp[:], sbw[:], prbb_t[:], start=True, stop=True)
                    pp = pppool.tile([128, 128], f32, tag="pp")
                    nc.scalar.activation(pp[:], psp[:], Copy, scale=npt[:, n:n + 1])
                    sim_m = simpool.tile([128, 64], f32, tag="simm")
                    nc.vector.scalar_tensor_tensor(
                        sim_m[:], pp[:, 0:64], tst[:, n:n + 1], pp[:, 64:128],
                        op0=mult, op1=sub)
                    obsim = obpool.tile([128, 512], f32, tag="ob")
                    nc.vector.scalar_tensor_tensor(
                        obsim[:].rearrange("p (s o) -> p s o", o=64),
                        pssp[:].rearrange("p (s o) -> p s o", o=64),
                        udt[:, n:n + 1],
                        sim_m[:].unsqueeze(1).broadcast_to([128, 8, 64]),
                        op0=mult, op1=add)
                    # real rows for l = l0+1 .. l0+128, lane p -> l0+p+1
                    rr = ropool.tile([128, 64], f32, tag="rr")
                    nc.vector.scalar_tensor_tensor(
                        rr[:], pp[:, 0:64], tsh[:, n:n + 1], pp[:, 64:128],
                        op0=mult, op1=sub)
                    rm = ropool.tile([128, 64], f32, tag="rm")
                    nc.vector.tensor_scalar_mul(rm[:], rr[:], nsh[:, n:n + 1])
                    # store
                    PR = 128 if n < NT - 1 else 127
                    real_dst = bass.AP(
                        OUTD.tensor, (b * ROWS + 9 * (n * 128 + 1)) * 64,
                        [[9 * 64, PR], [1, 64]])
                    nc.sync.dma_start(real_dst, rm[0:PR, :])
                    if n < NT - 1:
                        blk = OUTD[b, 9 * n * 128: 9 * (n + 1) * 128, :] \
                            .rearrange("(p s) o -> p s o", s=9)
                        nc.sync.dma_start(blk[:, 1:9, :],
                                          obsim[:].rearrange("p (s o) -> p s o", o=64))
                    else:
                        blk = OUTD[b, 9 * n * 128: 9 * n * 128 + 9 * 127, :] \
                            .rearrange("(p s) o -> p s o", s=9)
                        nc.sync.dma_start(
                            blk[:, 1:9, :],
                            obsim[0:127, :].rearrange("p (s o) -> p s o", o=64))
    nc.compile()
    return nc


_NC_CACHE = None


def kernel(**inputs):
    global _NC_CACHE
    from concourse.bass_utils import run_bass_kernel_spmd

    times = np.ascontiguousarray(inputs["times"], np.float32)
    feats = np.ascontiguousarray(inputs["features"], np.float32)
    npm = inputs["non_pad_mask"].astype(np.float32)
    u = np.asarray(inputs["uniform_sample"], np.float32)
    W = np.ascontiguousarray(inputs["W"], np.float32)
    bias = np.ascontiguousarray(inputs["bias_param"], np.float32)

    bandc, bandp, prba, prbb = _consts(W, bias, u)
    tnext = np.concatenate([times[:, 1:], np.zeros((B, 1), np.float32)], 1)
    npmn = np.concatenate([npm[:, 1:], np.zeros((B, 1), np.float32)], 1)
    udt = (tnext - times) * npm * npmn  # (B, L); l=L-1 col unused downstream

    if _NC_CACHE is None:
        _NC_CACHE = _build_nc()
    nc = _NC_CACHE

    pad = np.zeros((B, 128), np.float32)
    times_p = np.concatenate([times, pad], 1)
    npm_p = np.concatenate([npm, pad], 1)

    in_maps = []
    for c in range(NCORES):
        sl = slice(c * BPC, (c + 1) * BPC)
        in_maps.append({
            "f": np.ascontiguousarray(feats[sl]),
            "ts": np.ascontiguousarray(times_p[sl]),
            "ud": np.ascontiguousarray(udt[sl]),
            "np": np.ascontiguousarray(npm_p[sl]),
            "bandc": bandc, "bandp": bandp, "prba": prba, "prbb": prbb,
        })
    res = run_bass_kernel_spmd(nc, in_maps, core_ids=list(range(NCORES)))
    out = np.concatenate([r["out"] for r in res.results], 0)
    return out.astype(np.float32)



# revision 4
# speedup vs baseline: 1.7795x; 1.7795x over previous
"""Trainium2 Bass kernel for ContinuousConv1DSim (gnn_message_passing).

Reformulation (numpy-validated, rel err ~5e-4 in fp16):
  Times are re-centered per 8-event group (c_g = t[8g]) so the catastrophic
  cancellation t_l*A - D happens on small-magnitude quantities and the whole
  matmul chain runs in fp16:
    G_j   = f_j * npm_j                         (host, fp16)
    Hv1_j = (t_j - c_{g(j)})   * G_j            (host, fp16)
    Hv2_j = (t_j - c_{g(j)+1}) * G_j            (host, fp16)
  MM1 (PE, fp16, transposed out):  per 128-l tile
    psw[0:64]   (A)  += G^T  @ bandc   [+ G^T  @ bandp halo from prev tile]
    psw[64:128] (D') += Hv1^T @ bandSame + Hv2^T @ bandPrev [+ Hv2^T @ bandp]
    where bandSame/bandPrev split the causal 8-window by group equality,
    so D'[l] = sum_{j in win(l)} (t_j - c_{g(l)}) G_j exactly.
  MM2 (PE, fp16): psp = sbw^T @ prba = [A@Wt | D'@Wt - A@bias]
                  pssp = sbw^T @ prbb = [u_s * A@Wt]_s
  pp   = npm * psp                               (ACT scale-copy, fp32)
  simm = ppA*(t_l-c) - ppD                       (DVE STT)
  obsim= pssp*udt + simm                         (DVE STT, bcast over s)
  real = (ppA*(t_{l+1}-c) - ppD) * npm_{l+1}     (GpSimd STT + mul)
  One unified per-tile block [simm | obsim(512) | real(64)]; cols 64:640 are
  DMA'd as 576 contiguous fp16 per l directly into the output row layout
  (sim rows of l followed by real row of l+1), batched 4 tiles per DMA.
  Output DRAM is fp16 (row-padded); host casts to fp32 and zeroes row 0.

Pure data parallel: batch 32 -> 8 cores x 4. All params replicated.
"""

import numpy as np

B, L, C, O, S = 32, 2048, 64, 64, 8
NCORES = 8
BPC = B // NCORES          # 4 batches per core
NT = L // 128              # 16 l-tiles per batch
GS = 8                     # time-recentering group size
ROWS = (L - 1) * (S + 1) + 1   # 18424
RP = 9 * L + 8                 # padded rows in DRAM (last lane ends at 9*(L-1)+9)


def _consts(W, bias, u):
    n = np.arange(128)
    bandc = ((n[:, None] >= n[None, :] - 7) & (n[:, None] <= n[None, :]))
    bandp = (n[:, None] >= n[None, :] + 121)
    g = n // GS
    same = g[:, None] == g[None, :]
    prev = g[:, None] == g[None, :] - 1
    bandc_f = bandc.astype(np.float16)
    bandp_f = bandp.astype(np.float16)
    bandS_f = (bandc & same).astype(np.float16)
    bandP_f = (bandc & prev).astype(np.float16)
    Wt = W.T.astype(np.float16)           # (C, O)
    prba = np.zeros((128, 128), np.float16)
    prba[0:64, 0:64] = Wt                 # A@Wt
    prba[0:64, 64:128] = -bias.astype(np.float16)   # -A@bias into D''
    prba[64:128, 64:128] = Wt             # D'@Wt
    prbb = np.zeros((128, 512), np.float16)
    for s in range(S):
        prbb[0:64, s * 64:(s + 1) * 64] = (u[s] * W.T).astype(np.float16)
    return bandc_f, bandp_f, bandS_f, bandP_f, prba, prbb


def _host_prep(inputs):
    times = np.ascontiguousarray(inputs["times"], np.float32)
    feats = np.ascontiguousarray(inputs["features"], np.float32)
    npm = inputs["non_pad_mask"].astype(np.float32)
    u = np.asarray(inputs["uniform_sample"], np.float32)
    W = np.ascontiguousarray(inputs["W"], np.float32)
    bias = np.ascontiguousarray(inputs["bias_param"], np.float32)

    consts = _consts(W, bias, u)

    c = times[:, ::GS]                                   # (B, L/GS)
    c_of = np.repeat(c, GS, axis=1)                      # c_{g(l)}
    c_next = np.concatenate([c[:, 1:], c[:, -1:] + 1.0], axis=1)
    c_next_of = np.repeat(c_next, GS, axis=1)            # c_{g(l)+1}
    ttv1 = times - c_of
    ttv2 = times - c_next_of
    tnext = np.concatenate([times[:, 1:], np.zeros((B, 1), np.float32)], 1)
    npmn = np.concatenate([npm[:, 1:], np.zeros((B, 1), np.float32)], 1)
    udt = (tnext - times) * npm * npmn
    ttcR = tnext - c_of

    G = (feats * npm[:, :, None]).astype(np.float16)
    gh = np.empty((B, L, 192), np.float16)
    gh[:, :, 0:64] = G
    gh[:, :, 64:128] = G * ttv2[:, :, None].astype(np.float16)
    gh[:, :, 128:192] = G * ttv1[:, :, None].astype(np.float16)

    # per-lane scalars, partition-major: scal[b, p, n*5+k]
    # k: 0=npt 1=nsh 2=ttc 3=ttcR 4=udt
    sc = np.stack([npm, npmn, ttv1, ttcR, udt], axis=2)  # (B, L, 5)
    scal = np.ascontiguousarray(
        sc.reshape(B, NT, 128, 5).transpose(0, 2, 1, 3).reshape(B, 128, NT * 5)
    ).astype(np.float32)
    return gh, scal, consts


def _make_in_maps(inputs):
    gh, scal, consts = _host_prep(inputs)
    bandc, bandp, bandS, bandP, prba, prbb = consts
    in_maps = []
    for cidx in range(NCORES):
        sl = slice(cidx * BPC, (cidx + 1) * BPC)
        in_maps.append({
            "gh": np.ascontiguousarray(gh[sl]),
            "scal": np.ascontiguousarray(scal[sl]),
            "bandc": bandc, "bandp": bandp,
            "bandS": bandS, "bandP": bandP,
            "prba": prba, "prbb": prbb,
        })
    return in_maps


def _build_nc():
    import concourse.bass as bass
    import concourse.bacc as bacc
    import concourse.mybir as mybir
    import concourse.tile as tile

    f32 = mybir.dt.float32
    f16 = mybir.dt.float16
    Copy = mybir.ActivationFunctionType.Copy
    mult = mybir.AluOpType.mult
    sub = mybir.AluOpType.subtract
    add = mybir.AluOpType.add

    nc = bacc.Bacc("TRN2", target_bir_lowering=False, debug=False,
                   num_devices=NCORES)

    GHD = nc.dram_tensor("gh", [BPC, L, 192], f16, kind="ExternalInput").ap()
    SCD = nc.dram_tensor("scal", [BPC, 128, NT * 5], f32,
                         kind="ExternalInput").ap()
    BCD = nc.dram_tensor("bandc", [128, 128], f16, kind="ExternalInput").ap()
    BPD = nc.dram_tensor("bandp", [128, 128], f16, kind="ExternalInput").ap()
    BSD = nc.dram_tensor("bandS", [128, 128], f16, kind="ExternalInput").ap()
    BVD = nc.dram_tensor("bandP", [128, 128], f16, kind="ExternalInput").ap()
    PAD = nc.dram_tensor("prba", [128, 128], f16, kind="ExternalInput").ap()
    PBD = nc.dram_tensor("prbb", [128, 512], f16, kind="ExternalInput").ap()
    OUTD = nc.dram_tensor("out", [BPC, RP, O], f16, kind="ExternalOutput").ap()

    with tile.TileContext(nc) as tc:
        with (
            tc.tile_pool(name="const", bufs=1) as cpool,
            tc.tile_pool(name="ghp", bufs=2) as ghpool,
            tc.tile_pool(name="scp", bufs=2) as scpool,
            tc.tile_pool(name="sbw", bufs=3) as sbwpool,
            tc.tile_pool(name="pp", bufs=3) as pppool,
            tc.tile_pool(name="rr", bufs=3) as rrpool,
            tc.tile_pool(name="ob", bufs=2) as obpool,
            tc.tile_pool(name="psw", bufs=3, space=bass.MemorySpace.PSUM) as pwpool,
            tc.tile_pool(name="psp", bufs=2, space=bass.MemorySpace.PSUM) as papool,
            tc.tile_pool(name="pssp", bufs=2, space=bass.MemorySpace.PSUM) as pbpool,
        ):
            bandc_t = cpool.tile([128, 128], f16, tag="bandc")
            bandp_t = cpool.tile([128, 128], f16, tag="bandp")
            bandS_t = cpool.tile([128, 128], f16, tag="bandS")
            bandP_t = cpool.tile([128, 128], f16, tag="bandP")
            prba_t = cpool.tile([128, 128], f16, tag="prba")
            prbb_t = cpool.tile([128, 512], f16, tag="prbb")
            nc.sync.dma_start(bandc_t[:], BCD)
            nc.sync.dma_start(bandp_t[:], BPD)
            nc.sync.dma_start(bandS_t[:], BSD)
            nc.sync.dma_start(bandP_t[:], BVD)
            nc.sync.dma_start(prba_t[:], PAD)
            nc.sync.dma_start(prbb_t[:], PBD)

            for b in range(BPC):
                gha = ghpool.tile([128, NT, 192], f16, tag="gha")
                nc.sync.dma_start(
                    gha[:], GHD[b].rearrange("(n p) c -> p n c", p=128))
                scal = scpool.tile([128, NT * 5], f32, tag="scal")
                nc.scalar.dma_start(scal[:], SCD[b])

                psw_next = None
                bigg = None
                for n in range(NT):
                    gh = gha[:, n, :]
                    # ---- MM1: windowed sums, transposed out ----
                    if n == 0:
                        psw = pwpool.tile([128, 128], f32, tag="psw")
                        nc.tensor.matmul(psw[0:64, :], gh[:, 0:64], bandc_t[:],
                                         start=True, stop=True)
                        nc.tensor.matmul(psw[64:128, :], gh[:, 128:192],
                                         bandS_t[:], start=True, stop=False,
                                         tile_position=(0, 64))
                        nc.tensor.matmul(psw[64:128, :], gh[:, 64:128],
                                         bandP_t[:], start=False, stop=True,
                                         tile_position=(0, 64))
                    else:
                        psw = psw_next
                        nc.tensor.matmul(psw[0:64, :], gh[:, 0:64], bandc_t[:],
                                         start=False, stop=True)
                        nc.tensor.matmul(psw[64:128, :], gh[:, 128:192],
                                         bandS_t[:], start=False, stop=False,
                                         tile_position=(0, 64))
                        nc.tensor.matmul(psw[64:128, :], gh[:, 64:128],
                                         bandP_t[:], start=False, stop=True,
                                         tile_position=(0, 64))
                    if n < NT - 1:
                        psw_next = pwpool.tile([128, 128], f32, tag="psw")
                        nc.tensor.matmul(psw_next[:], gh[:, 0:128], bandp_t[:],
                                         start=True, stop=False)
                    # ---- MM2 ----
                    sbw = sbwpool.tile([128, 128], f16, tag="sbw")
                    nc.scalar.copy(sbw[:], psw[:])
                    psp = papool.tile([128, 128], f32, tag="psp")
                    nc.tensor.matmul(psp[:], sbw[:], prba_t[:],
                                     start=True, stop=True)
                    pssp = pbpool.tile([128, 512], f32, tag="pssp")
                    nc.tensor.matmul(pssp[:], sbw[:], prbb_t[:],
                                     start=True, stop=True)
                    # ---- elementwise + output assembly ----
                    pp = pppool.tile([128, 128], f32, tag="pp")
                    nc.scalar.activation(pp[:], psp[:], Copy,
                                         scale=scal[:, 5 * n:5 * n + 1])
                    if n % 4 == 0:
                        bigg = obpool.tile([128, 4 * 640], f16, tag="bigg")
                    sub_t = bigg[:, (n % 4) * 640:(n % 4 + 1) * 640]
                    simm = sub_t[:, 0:64]
                    nc.vector.scalar_tensor_tensor(
                        simm, pp[:, 0:64], scal[:, 5 * n + 2:5 * n + 3],
                        pp[:, 64:128], op0=mult, op1=sub)
                    nc.vector.scalar_tensor_tensor(
                        sub_t[:, 64:576].rearrange("p (s o) -> p s o", o=64),
                        pssp[:].rearrange("p (s o) -> p s o", o=64),
                        scal[:, 5 * n + 4:5 * n + 5],
                        simm.unsqueeze(1).broadcast_to([128, 8, 64]),
                        op0=mult, op1=add)
                    rr = rrpool.tile([128, 64], f16, tag="rr")
                    nc.vector.scalar_tensor_tensor(
                        rr[:], pp[:, 0:64], scal[:, 5 * n + 3:5 * n + 4],
                        pp[:, 64:128], op0=mult, op1=sub)
                    nc.vector.tensor_scalar_mul(
                        sub_t[:, 576:640], rr[:],
                        scal[:, 5 * n + 1:5 * n + 2])
                    # ---- store: 4 tiles per DMA ----
                    if n % 4 == 3:
                        n0 = n - 3
                        dst = bass.AP(
                            OUTD.tensor,
                            b * RP * 64 + (9 * n0 * 128 + 1) * 64,
                            [[576, 128], [9 * 128 * 64, 4], [1, 576]])
                        src = bigg[:].rearrange(
                            "p (j c) -> p j c", c=640)[:, :, 64:640]
                        nc.sync.dma_start(dst, src)
    nc.compile()
    return nc


_NC_CACHE = None


def kernel(**inputs):
    global _NC_CACHE
    from concourse.bass_utils import run_bass_kernel_spmd

    in_maps = _make_in_maps(inputs)
    if _NC_CACHE is None:
        _NC_CACHE = _build_nc()
    nc = _NC_CACHE

    res = run_bass_kernel_spmd(nc, in_maps, core_ids=list(range(NCORES)))
    out = np.concatenate([r["out"] for r in res.results], 0)
    out = out[:, :ROWS, :].astype(np.float32)
    out[:, 0, :] = 0.0
    return out


# revision 8
# speedup vs baseline: 2.3547x; 1.3232x over previous
"""Trainium2 Bass kernel for ContinuousConv1DSim (gnn_message_passing).

Reformulation (numpy-validated, rel err ~2.4e-3 in fp16):
  Times are re-centered per 128-event tile (C_n = t[128n+64]) so the
  cancellation t_l*A - D happens on small-magnitude quantities and the whole
  matmul chain runs in fp16:
    G_j   = f_j * npm_j              Hv1_j = (t_j - C_{n(j)})   * G_j
    Hv2_j = (t_j - C_{n(j)+1}) * G_j                      (all host, fp16)
  MM1 (PE, fp16, transposed out), per 128-l tile, psw = [A | D] channels:
    main: psw      += [G|Hv1]^T @ bandc      (one 128-wide matmul)
    halo: psw_next  = [G|Hv2]^T @ bandp      (one matmul, 2-chunk lhsT AP)
  MM2 (PE, fp16): psp = sbw^T @ prba = [A@Wt | D@Wt - A@bias]
                  pssp = sbw^T @ prbb = [u_s * A@Wt]_s
  pp   = npm * psp                  (ACT scale-copy, fp32 - keeps cancellation)
  simm = ppA*(t_l-C) - ppD          (DVE STT)
  obsim= pssp*udt + simm            (DVE STT, bcast over s)
  rr   = ppA*(t_{l+1}-C) - ppD      (DVE STT);  real = rr * npm_{l+1} (ACT)
  Per-tile block [simm | obsim(512) | real(64)]; cols 64:640 are DMA'd as 576
  contiguous fp16 per l into the output row layout (sim rows of l then real
  row of l+1), batched 4 tiles per DMA.  Output DRAM is fp16 (row-padded);
  host casts to fp32 and zeroes row 0.
  The loop is software-pipelined one tile deep: PE/ACT producer work for tile
  n issues before DVE/store work for tile n-1, so engines overlap.

Pure data parallel: batch 32 -> 8 cores x 4. All params replicated.
"""

import numpy as np

B, L, C, O, S = 32, 2048, 64, 64, 8
NCORES = 8
BPC = B // NCORES          # 4 batches per core
NT = L // 128              # 16 l-tiles per batch
ROWS = (L - 1) * (S + 1) + 1   # 18424
RP = 9 * L + 8                 # padded rows in DRAM (last lane ends at 9*(L-1)+9)


def _consts(W, bias, u):
    n = np.arange(128)
    bandc = ((n[:, None] >= n[None, :] - 7) & (n[:, None] <= n[None, :]))
    bandp = (n[:, None] >= n[None, :] + 121)
    Wt = W.T.astype(np.float16)           # (C, O)
    prba = np.zeros((128, 128), np.float16)
    prba[0:64, 0:64] = Wt                 # A@Wt
    prba[0:64, 64:128] = -bias.astype(np.float16)   # -A@bias into D''
    prba[64:128, 64:128] = Wt             # D@Wt
    prbb = np.zeros((128, 512), np.float16)
    for s in range(S):
        prbb[0:64, s * 64:(s + 1) * 64] = (u[s] * W.T).astype(np.float16)
    return (bandc.astype(np.float16), bandp.astype(np.float16), prba, prbb)


def _host_prep(inputs):
    times = np.ascontiguousarray(inputs["times"], np.float32)
    feats = np.ascontiguousarray(inputs["features"], np.float32)
    npm = inputs["non_pad_mask"].astype(np.float32)
    u = np.asarray(inputs["uniform_sample"], np.float32)
    W = np.ascontiguousarray(inputs["W"], np.float32)
    bias = np.ascontiguousarray(inputs["bias_param"], np.float32)

    consts = _consts(W, bias, u)

    Cn = times[:, 64::128]                               # (B, NT) tile centers
    C_of = np.repeat(Cn, 128, axis=1)                    # C_{n(l)}
    C_next = np.concatenate([Cn[:, 1:], Cn[:, -1:] + 1.0], axis=1)
    C_next_of = np.repeat(C_next, 128, axis=1)
    tnext = np.concatenate([times[:, 1:], np.zeros((B, 1), np.float32)], 1)
    npmn = np.concatenate([npm[:, 1:], np.zeros((B, 1), np.float32)], 1)
    udt = (tnext - times) * npm * npmn
    ttc = times - C_of
    ttcR = tnext - C_of

    G = (feats * npm[:, :, None]).astype(np.float16)
    gh = np.empty((B, L, 256), np.float16)
    gh[:, :, 0:64] = G
    gh[:, :, 64:128] = G * ttc[:, :, None].astype(np.float16)        # Hv1
    gh[:, :, 128:192] = G
    gh[:, :, 192:256] = G * (times - C_next_of)[:, :, None].astype(np.float16)

    # per-lane scalars, partition-major: scal[b, p, n*5+k]
    # k: 0=npt 1=nsh 2=ttc 3=ttcR 4=udt
    sc = np.stack([npm, npmn, ttc, ttcR, udt], axis=2)   # (B, L, 5)
    scal = np.ascontiguousarray(
        sc.reshape(B, NT, 128, 5).transpose(0, 2, 1, 3).reshape(B, 128, NT * 5)
    ).astype(np.float32)
    return gh, scal, consts


def _make_in_maps(inputs):
    gh, scal, consts = _host_prep(inputs)
    bandc, bandp, prba, prbb = consts
    in_maps = []
    for cidx in range(NCORES):
        sl = slice(cidx * BPC, (cidx + 1) * BPC)
        in_maps.append({
            "gh": np.ascontiguousarray(gh[sl]),
            "scal": np.ascontiguousarray(scal[sl]),
            "bandc": bandc, "bandp": bandp,
            "prba": prba, "prbb": prbb,
        })
    return in_maps


def _build_nc():
    import concourse.bass as bass
    import concourse.bacc as bacc
    import concourse.mybir as mybir
    import concourse.tile as tile

    f32 = mybir.dt.float32
    f16 = mybir.dt.float16
    Copy = mybir.ActivationFunctionType.Copy
    mult = mybir.AluOpType.mult
    sub = mybir.AluOpType.subtract
    add = mybir.AluOpType.add

    nc = bacc.Bacc("TRN2", target_bir_lowering=False, debug=False,
                   num_devices=NCORES)

    GHD = nc.dram_tensor("gh", [BPC, L, 256], f16, kind="ExternalInput").ap()
    SCD = nc.dram_tensor("scal", [BPC, 128, NT * 5], f32,
                         kind="ExternalInput").ap()
    BCD = nc.dram_tensor("bandc", [128, 128], f16, kind="ExternalInput").ap()
    BPD = nc.dram_tensor("bandp", [128, 128], f16, kind="ExternalInput").ap()
    PAD = nc.dram_tensor("prba", [128, 128], f16, kind="ExternalInput").ap()
    PBD = nc.dram_tensor("prbb", [128, 512], f16, kind="ExternalInput").ap()
    OUTD = nc.dram_tensor("out", [BPC, RP, O], f16, kind="ExternalOutput").ap()

    with tile.TileContext(nc) as tc:
        with (
            tc.tile_pool(name="const", bufs=1) as cpool,
            tc.tile_pool(name="ghp", bufs=2) as ghpool,
            tc.tile_pool(name="scp", bufs=2) as scpool,
            tc.tile_pool(name="sbw", bufs=3) as sbwpool,
            tc.tile_pool(name="pp", bufs=3) as pppool,
            tc.tile_pool(name="rr", bufs=3) as rrpool,
            tc.tile_pool(name="ob", bufs=2) as obpool,
            tc.tile_pool(name="psw", bufs=3, space=bass.MemorySpace.PSUM) as pwpool,
            tc.tile_pool(name="psp", bufs=2, space=bass.MemorySpace.PSUM) as papool,
            tc.tile_pool(name="pssp", bufs=2, space=bass.MemorySpace.PSUM) as pbpool,
        ):
            bandc_t = cpool.tile([128, 128], f16, tag="bandc")
            bandp_t = cpool.tile([128, 128], f16, tag="bandp")
            prba_t = cpool.tile([128, 128], f16, tag="prba")
            prbb_t = cpool.tile([128, 512], f16, tag="prbb")
            nc.sync.dma_start(bandc_t[:], BCD)
            nc.sync.dma_start(bandp_t[:], BPD)
            nc.sync.dma_start(prba_t[:], PAD)
            nc.sync.dma_start(prbb_t[:], PBD)

            for b in range(BPC):
                gha = ghpool.tile([128, NT, 256], f16, tag="gha")
                nc.sync.dma_start(
                    gha[:], GHD[b].rearrange("(n p) c -> p n c", p=128))
                scal = scpool.tile([128, NT * 5], f32, tag="scal")
                nc.scalar.dma_start(scal[:], SCD[b])

                psw_next = None
                biggs = {}
                pps = {}
                pssps = {}

                def stage_a(n, psw_next_in):
                    gh = gha[:, n, :]
                    if n == 0:
                        psw = pwpool.tile([128, 128], f32, tag="psw")
                        nc.tensor.matmul(psw[:], gh[:, 0:128], bandc_t[:],
                                         start=True, stop=True)
                    else:
                        psw = psw_next_in
                        nc.tensor.matmul(psw[:], gh[:, 0:128], bandc_t[:],
                                         start=False, stop=True)
                    nxt = None
                    if n < NT - 1:
                        nxt = pwpool.tile([128, 128], f32, tag="psw")
                        nc.tensor.matmul(nxt[:], gh[:, 128:256], bandp_t[:],
                                         start=True, stop=False)
                    sbw = sbwpool.tile([128, 128], f16, tag="sbw")
                    nc.scalar.copy(sbw[:], psw[:])
                    psp = papool.tile([128, 128], f32, tag="psp")
                    nc.tensor.matmul(psp[:], sbw[:], prba_t[:],
                                     start=True, stop=True)
                    pssp = pbpool.tile([128, 512], f32, tag="pssp")
                    nc.tensor.matmul(pssp[:], sbw[:], prbb_t[:],
                                     start=True, stop=True)
                    pp = pppool.tile([128, 128], f32, tag="pp")
                    nc.scalar.activation(pp[:], psp[:], Copy,
                                         scale=scal[:, 5 * n:5 * n + 1])
                    pps[n] = pp
                    pssps[n] = pssp
                    return nxt

                def stage_b(m):
                    pp = pps.pop(m)
                    pssp = pssps.pop(m)
                    if m % 4 == 0:
                        biggs[m // 4] = obpool.tile([128, 4 * 640], f16,
                                                    name="bigg", tag="bigg")
                    bigg = biggs[m // 4]
                    sub_t = bigg[:, (m % 4) * 640:(m % 4 + 1) * 640]
                    simm = sub_t[:, 0:64]
                    nc.vector.scalar_tensor_tensor(
                        simm, pp[:, 0:64], scal[:, 5 * m + 2:5 * m + 3],
                        pp[:, 64:128], op0=mult, op1=sub)
                    nc.vector.scalar_tensor_tensor(
                        sub_t[:, 64:576].rearrange("p (s o) -> p s o", o=64),
                        pssp[:].rearrange("p (s o) -> p s o", o=64),
                        scal[:, 5 * m + 4:5 * m + 5],
                        simm.unsqueeze(1).broadcast_to([128, 8, 64]),
                        op0=mult, op1=add)
                    rr = rrpool.tile([128, 64], f16, tag="rr")
                    nc.vector.scalar_tensor_tensor(
                        rr[:], pp[:, 0:64], scal[:, 5 * m + 3:5 * m + 4],
                        pp[:, 64:128], op0=mult, op1=sub)
                    nc.scalar.activation(sub_t[:, 576:640], rr[:], Copy,
                                         scale=scal[:, 5 * m + 1:5 * m + 2])
                    if m % 4 == 3:
                        n0 = m - 3
                        dst = bass.AP(
                            OUTD.tensor,
                            b * RP * 64 + (9 * n0 * 128 + 1) * 64,
                            [[576, 128], [9 * 128 * 64, 4], [1, 576]])
                        src = biggs.pop(m // 4)[:].rearrange(
                            "p (j c) -> p j c", c=640)[:, :, 64:640]
                        nc.sync.dma_start(dst, src)

                for n in range(NT):
                    psw_next = stage_a(n, psw_next)
                    if n >= 1:
                        stage_b(n - 1)
                stage_b(NT - 1)
    nc.compile()
    return nc


_NC_CACHE = None


def kernel(**inputs):
    global _NC_CACHE
    from concourse.bass_utils import run_bass_kernel_spmd

    in_maps = _make_in_maps(inputs)
    if _NC_CACHE is None:
        _NC_CACHE = _build_nc()
    nc = _NC_CACHE

    res = run_bass_kernel_spmd(nc, in_maps, core_ids=list(range(NCORES)))
    out = np.concatenate([r["out"] for r in res.results], 0)
    out = out[:, :ROWS, :].astype(np.float32)
    out[:, 0, :] = 0.0
    return out


# revision 9
# speedup vs baseline: 2.6511x; 1.1259x over previous
"""Trainium2 Bass kernel for ContinuousConv1DSim (gnn_message_passing).

Reformulation (numpy-validated, rel err ~2.4e-3 in fp16):
  Times are re-centered per 128-event tile (C_n = t[128n+64]) so the
  cancellation t_l*A - D happens on small-magnitude quantities and the whole
  matmul chain runs in fp16:
    G_j   = f_j * npm_j              Hv1_j = (t_j - C_{n(j)})   * G_j
    Hv2_j = (t_j - C_{n(j)+1}) * G_j                      (all host, fp16)
  MM1 (PE, fp16, transposed out), per 128-l tile, psw = [A | D] channels:
    main: psw      += [G|Hv1]^T @ bandc      (one 128-wide matmul)
    halo: psw_next  = [G|Hv2]^T @ bandp      (one matmul, 2-chunk lhsT AP)
  MM2 (PE, fp16): psp = sbw^T @ prba = [A@Wt | D@Wt - A@bias]
                  pssp = sbw^T @ prbb = [u_s * A@Wt]_s
  pp   = npm * psp                  (ACT scale-copy, fp32 - keeps cancellation)
  simm = ppA*(t_l-C) - ppD          (DVE STT)
  obsim= pssp*udt + simm            (DVE STT, bcast over s)
  rr   = ppA*(t_{l+1}-C) - ppD      (DVE STT);  real = rr * npm_{l+1} (ACT)
  Per-tile block [simm | obsim(512) | real(64)]; cols 64:640 are DMA'd as 576
  contiguous fp16 per l into the output row layout (sim rows of l then real
  row of l+1), batched 4 tiles per DMA.  Output DRAM is fp16 (row-padded);
  host casts to fp32 and zeroes row 0.
  The loop is software-pipelined one tile deep: PE/ACT producer work for tile
  n issues before DVE/store work for tile n-1, so engines overlap.

Pure data parallel: batch 32 -> 8 cores x 4. All params replicated.
"""

import numpy as np

B, L, C, O, S = 32, 2048, 64, 64, 8
NCORES = 8
BPC = B // NCORES          # 4 batches per core
NT = L // 128              # 16 l-tiles per batch
ROWS = (L - 1) * (S + 1) + 1   # 18424
RP = 9 * L + 8                 # padded rows in DRAM (last lane ends at 9*(L-1)+9)


def _consts(W, bias, u):
    n = np.arange(128)
    bandc = ((n[:, None] >= n[None, :] - 7) & (n[:, None] <= n[None, :]))
    bandp = (n[:, None] >= n[None, :] + 121)
    Wt = W.T.astype(np.float16)           # (C, O)
    prba = np.zeros((128, 128), np.float16)
    prba[0:64, 0:64] = Wt                 # A@Wt
    prba[0:64, 64:128] = -bias.astype(np.float16)   # -A@bias into D''
    prba[64:128, 64:128] = Wt             # D@Wt
    prbb = np.zeros((128, 512), np.float16)
    for s in range(S):
        prbb[0:64, s * 64:(s + 1) * 64] = (u[s] * W.T).astype(np.float16)
    return (bandc.astype(np.float16), bandp.astype(np.float16), prba, prbb)


def _host_prep(inputs):
    times = np.ascontiguousarray(inputs["times"], np.float32)
    feats = np.ascontiguousarray(inputs["features"], np.float32)
    npm = inputs["non_pad_mask"].astype(np.float32)
    u = np.asarray(inputs["uniform_sample"], np.float32)
    W = np.ascontiguousarray(inputs["W"], np.float32)
    bias = np.ascontiguousarray(inputs["bias_param"], np.float32)

    consts = _consts(W, bias, u)

    Cn = times[:, 64::128]                               # (B, NT) tile centers
    C_of = np.repeat(Cn, 128, axis=1)                    # C_{n(l)}
    C_next = np.concatenate([Cn[:, 1:], Cn[:, -1:] + 1.0], axis=1)
    C_next_of = np.repeat(C_next, 128, axis=1)
    tnext = np.concatenate([times[:, 1:], np.zeros((B, 1), np.float32)], 1)
    npmn = np.concatenate([npm[:, 1:], np.zeros((B, 1), np.float32)], 1)
    udt = (tnext - times) * npm * npmn
    ttc = times - C_of
    ttcR = tnext - C_of

    G = (feats * npm[:, :, None]).astype(np.float16)
    gh = np.empty((B, L, 256), np.float16)
    gh[:, :, 0:64] = G
    gh[:, :, 64:128] = G * ttc[:, :, None].astype(np.float16)        # Hv1
    gh[:, :, 128:192] = G
    gh[:, :, 192:256] = G * (times - C_next_of)[:, :, None].astype(np.float16)

    # per-lane scalars, partition-major: scal[b, p, n*5+k]
    # k: 0=npt 1=nsh 2=ttc 3=ttcR 4=udt
    sc = np.stack([npm, npmn, ttc, ttcR, udt], axis=2)   # (B, L, 5)
    scal = np.ascontiguousarray(
        sc.reshape(B, NT, 128, 5).transpose(0, 2, 1, 3).reshape(B, 128, NT * 5)
    ).astype(np.float32)
    return gh, scal, consts


def _make_in_maps(inputs):
    gh, scal, consts = _host_prep(inputs)
    bandc, bandp, prba, prbb = consts
    in_maps = []
    for cidx in range(NCORES):
        sl = slice(cidx * BPC, (cidx + 1) * BPC)
        in_maps.append({
            "gh": np.ascontiguousarray(gh[sl]),
            "scal": np.ascontiguousarray(scal[sl]),
            "bandc": bandc, "bandp": bandp,
            "prba": prba, "prbb": prbb,
        })
    return in_maps


def _build_nc():
    import concourse.bass as bass
    import concourse.bacc as bacc
    import concourse.mybir as mybir
    import concourse.tile as tile

    f32 = mybir.dt.float32
    f16 = mybir.dt.float16
    Copy = mybir.ActivationFunctionType.Copy
    mult = mybir.AluOpType.mult
    sub = mybir.AluOpType.subtract
    add = mybir.AluOpType.add

    nc = bacc.Bacc("TRN2", target_bir_lowering=False, debug=False,
                   num_devices=NCORES)

    GHD = nc.dram_tensor("gh", [BPC, L, 256], f16, kind="ExternalInput").ap()
    SCD = nc.dram_tensor("scal", [BPC, 128, NT * 5], f32,
                         kind="ExternalInput").ap()
    BCD = nc.dram_tensor("bandc", [128, 128], f16, kind="ExternalInput").ap()
    BPD = nc.dram_tensor("bandp", [128, 128], f16, kind="ExternalInput").ap()
    PAD = nc.dram_tensor("prba", [128, 128], f16, kind="ExternalInput").ap()
    PBD = nc.dram_tensor("prbb", [128, 512], f16, kind="ExternalInput").ap()
    OUTD = nc.dram_tensor("out", [BPC, RP, O], f16, kind="ExternalOutput").ap()

    with tile.TileContext(nc) as tc:
        with (
            tc.tile_pool(name="const", bufs=1) as cpool,
            tc.tile_pool(name="ghp", bufs=8) as ghpool,
            tc.tile_pool(name="scp", bufs=2) as scpool,
            tc.tile_pool(name="sbw", bufs=3) as sbwpool,
            tc.tile_pool(name="pp", bufs=3) as pppool,
            tc.tile_pool(name="rr", bufs=3) as rrpool,
            tc.tile_pool(name="ob", bufs=3) as obpool,
            tc.tile_pool(name="psw", bufs=3, space=bass.MemorySpace.PSUM) as pwpool,
            tc.tile_pool(name="psp", bufs=2, space=bass.MemorySpace.PSUM) as papool,
            tc.tile_pool(name="pssp", bufs=2, space=bass.MemorySpace.PSUM) as pbpool,
        ):
            bandc_t = cpool.tile([128, 128], f16, tag="bandc")
            bandp_t = cpool.tile([128, 128], f16, tag="bandp")
            prba_t = cpool.tile([128, 128], f16, tag="prba")
            prbb_t = cpool.tile([128, 512], f16, tag="prbb")

            ghcs = {}      # (b, chunk) -> tile
            scals = {}     # b -> tile

            def load_gh_chunk(b, j):
                ghc = ghpool.tile([128, 4, 256], f16, name="ghc", tag="ghc")
                nc.sync.dma_start(
                    ghc[:],
                    GHD[b].rearrange("(n p) c -> p n c", p=128)[:, 4 * j:4 * j + 4, :])
                ghcs[(b, j)] = ghc

            def load_scal(b):
                sct = scpool.tile([128, NT * 5], f32, name="sct", tag="sct")
                nc.scalar.dma_start(sct[:], SCD[b])
                scals[b] = sct

            # first chunk of batch 0 gates everything - issue it first
            load_gh_chunk(0, 0)
            load_scal(0)
            nc.scalar.dma_start(bandc_t[:], BCD)
            nc.scalar.dma_start(bandp_t[:], BPD)
            nc.scalar.dma_start(prba_t[:], PAD)
            nc.scalar.dma_start(prbb_t[:], PBD)
            for j in range(1, 4):
                load_gh_chunk(0, j)

            for b in range(BPC):
                scal = scals.pop(b)

                psw_next = None
                biggs = {}
                pps = {}
                pssps = {}

                def stage_a(n, psw_next_in):
                    gh = ghcs[(b, n // 4)][:, n % 4, :]
                    if n == 0:
                        psw = pwpool.tile([128, 128], f32, tag="psw")
                        nc.tensor.matmul(psw[:], gh[:, 0:128], bandc_t[:],
                                         start=True, stop=True)
                    else:
                        psw = psw_next_in
                        nc.tensor.matmul(psw[:], gh[:, 0:128], bandc_t[:],
                                         start=False, stop=True)
                    nxt = None
                    if n < NT - 1:
                        nxt = pwpool.tile([128, 128], f32, tag="psw")
                        nc.tensor.matmul(nxt[:], gh[:, 128:256], bandp_t[:],
                                         start=True, stop=False)
                    sbw = sbwpool.tile([128, 128], f16, tag="sbw")
                    nc.scalar.copy(sbw[:], psw[:])
                    psp = papool.tile([128, 128], f32, tag="psp")
                    nc.tensor.matmul(psp[:], sbw[:], prba_t[:],
                                     start=True, stop=True)
                    pssp = pbpool.tile([128, 512], f32, tag="pssp")
                    nc.tensor.matmul(pssp[:], sbw[:], prbb_t[:],
                                     start=True, stop=True)
                    pp = pppool.tile([128, 128], f32, tag="pp")
                    nc.scalar.activation(pp[:], psp[:], Copy,
                                         scale=scal[:, 5 * n:5 * n + 1])
                    pps[n] = pp
                    pssps[n] = pssp
                    return nxt

                def stage_b(m):
                    pp = pps.pop(m)
                    pssp = pssps.pop(m)
                    if m % 4 == 0:
                        biggs[m // 4] = obpool.tile([128, 4 * 640], f16,
                                                    name="bigg", tag="bigg")
                    bigg = biggs[m // 4]
                    sub_t = bigg[:, (m % 4) * 640:(m % 4 + 1) * 640]
                    simm = sub_t[:, 0:64]
                    nc.vector.scalar_tensor_tensor(
                        simm, pp[:, 0:64], scal[:, 5 * m + 2:5 * m + 3],
                        pp[:, 64:128], op0=mult, op1=sub)
                    nc.vector.scalar_tensor_tensor(
                        sub_t[:, 64:576].rearrange("p (s o) -> p s o", o=64),
                        pssp[:].rearrange("p (s o) -> p s o", o=64),
                        scal[:, 5 * m + 4:5 * m + 5],
                        simm.unsqueeze(1).broadcast_to([128, 8, 64]),
                        op0=mult, op1=add)
                    rr = rrpool.tile([128, 64], f16, tag="rr")
                    nc.vector.scalar_tensor_tensor(
                        rr[:], pp[:, 0:64], scal[:, 5 * m + 3:5 * m + 4],
                        pp[:, 64:128], op0=mult, op1=sub)
                    nc.scalar.activation(sub_t[:, 576:640], rr[:], Copy,
                                         scale=scal[:, 5 * m + 1:5 * m + 2])
                    if m % 4 == 3:
                        n0 = m - 3
                        dst = bass.AP(
                            OUTD.tensor,
                            b * RP * 64 + (9 * n0 * 128 + 1) * 64,
                            [[576, 128], [9 * 128 * 64, 4], [1, 576]])
                        src = biggs.pop(m // 4)[:].rearrange(
                            "p (j c) -> p j c", c=640)[:, :, 64:640]
                        nc.sync.dma_start(dst, src)

                for n in range(NT):
                    psw_next = stage_a(n, psw_next)
                    if b + 1 < BPC:
                        if n == 1:
                            load_scal(b + 1)
                        if n % 4 == 2:
                            load_gh_chunk(b + 1, n // 4)
                    if n >= 1:
                        stage_b(n - 1)
                    if n % 4 == 3:
                        ghcs.pop((b, n // 4))
                stage_b(NT - 1)
    nc.compile()
    return nc


_NC_CACHE = None


def kernel(**inputs):
    global _NC_CACHE
    from concourse.bass_utils import run_bass_kernel_spmd

    in_maps = _make_in_maps(inputs)
    if _NC_CACHE is None:
        _NC_CACHE = _build_nc()
    nc = _NC_CACHE

    res = run_bass_kernel_spmd(nc, in_maps, core_ids=list(range(NCORES)))
    out = np.concatenate([r["out"] for r in res.results], 0)
    out = out[:, :ROWS, :].astype(np.float32)
    out[:, 0, :] = 0.0
    return out


# revision 10
# speedup vs baseline: 2.8123x; 1.0608x over previous
"""Trainium2 Bass kernel for ContinuousConv1DSim (gnn_message_passing).

Reformulation (numpy-validated, rel err ~2.4e-3 in fp16):
  Times are re-centered per 128-event tile (C_n = t[128n+64]) so the
  cancellation t_l*A - D happens on small-magnitude quantities and the whole
  matmul chain runs in fp16:
    G_j   = f_j * npm_j              Hv1_j = (t_j - C_{n(j)})   * G_j
    Hv2_j = (t_j - C_{n(j)+1}) * G_j                      (all host, fp16)
  MM1 (PE, fp16, transposed out), per 128-l tile, psw = [A | D] channels:
    main: psw      += [G|Hv1]^T @ bandc      (one 128-wide matmul)
    halo: psw_next  = [G|Hv2]^T @ bandp      (one matmul, 2-chunk lhsT AP)
  MM2 (PE, fp16): psp = sbw^T @ prba = [A@Wt | D@Wt - A@bias]
                  pssp = sbw^T @ prbb = [u_s * A@Wt]_s
  pp   = npm * psp                  (ACT scale-copy, fp32 - keeps cancellation)
  simm = ppA*(t_l-C) - ppD          (DVE STT)
  obsim= pssp*udt + simm            (DVE STT, bcast over s)
  rr   = ppA*(t_{l+1}-C) - ppD      (DVE STT);  real = rr * npm_{l+1} (ACT)
  Per-tile block [simm | obsim(512) | real(64)]; cols 64:640 are DMA'd as 576
  contiguous fp16 per l into the output row layout (sim rows of l then real
  row of l+1), batched 4 tiles per DMA.  Output DRAM is fp16 (row-padded);
  host casts to fp32 and zeroes row 0.
  The loop is software-pipelined one tile deep: PE/ACT producer work for tile
  n issues before DVE/store work for tile n-1, so engines overlap.

Pure data parallel: batch 32 -> 8 cores x 4. All params replicated.
"""

import numpy as np

B, L, C, O, S = 32, 2048, 64, 64, 8
NCORES = 8
BPC = B // NCORES          # 4 batches per core
NT = L // 128              # 16 l-tiles per batch
ROWS = (L - 1) * (S + 1) + 1   # 18424
RP = 9 * L + 8                 # padded rows in DRAM (last lane ends at 9*(L-1)+9)


def _consts(W, bias, u):
    n = np.arange(128)
    bandc = ((n[:, None] >= n[None, :] - 7) & (n[:, None] <= n[None, :]))
    bandp = (n[:, None] >= n[None, :] + 121)
    Wt = W.T.astype(np.float16)           # (C, O)
    prba = np.zeros((128, 128), np.float16)
    prba[0:64, 0:64] = Wt                 # A@Wt
    prba[0:64, 64:128] = -bias.astype(np.float16)   # -A@bias into D''
    prba[64:128, 64:128] = Wt             # D@Wt
    prbb = np.zeros((128, 512), np.float16)
    for s in range(S):
        prbb[0:64, s * 64:(s + 1) * 64] = (u[s] * W.T).astype(np.float16)
    return (bandc.astype(np.float16), bandp.astype(np.float16), prba, prbb)


def _host_prep(inputs):
    times = np.ascontiguousarray(inputs["times"], np.float32)
    feats = np.ascontiguousarray(inputs["features"], np.float32)
    npm = inputs["non_pad_mask"].astype(np.float32)
    u = np.asarray(inputs["uniform_sample"], np.float32)
    W = np.ascontiguousarray(inputs["W"], np.float32)
    bias = np.ascontiguousarray(inputs["bias_param"], np.float32)

    consts = _consts(W, bias, u)

    Cn = times[:, 64::128]                               # (B, NT) tile centers
    C_of = np.repeat(Cn, 128, axis=1)                    # C_{n(l)}
    C_next = np.concatenate([Cn[:, 1:], Cn[:, -1:] + 1.0], axis=1)
    C_next_of = np.repeat(C_next, 128, axis=1)
    tnext = np.concatenate([times[:, 1:], np.zeros((B, 1), np.float32)], 1)
    npmn = np.concatenate([npm[:, 1:], np.zeros((B, 1), np.float32)], 1)
    udt = (tnext - times) * npm * npmn
    ttc = times - C_of
    ttcR = tnext - C_of

    G = (feats * npm[:, :, None]).astype(np.float16)
    gh = np.empty((B, L, 256), np.float16)
    gh[:, :, 0:64] = G
    gh[:, :, 64:128] = G * ttc[:, :, None].astype(np.float16)        # Hv1
    gh[:, :, 128:192] = G
    gh[:, :, 192:256] = G * (times - C_next_of)[:, :, None].astype(np.float16)

    # per-lane scalars, partition-major: scal[b, p, n*5+k]
    # k: 0=npt 1=nsh 2=ttc 3=ttcR 4=udt
    sc = np.stack([npm, npmn, ttc, ttcR, udt], axis=2)   # (B, L, 5)
    scal = np.ascontiguousarray(
        sc.reshape(B, NT, 128, 5).transpose(0, 2, 1, 3).reshape(B, 128, NT * 5)
    ).astype(np.float32)
    return gh, scal, consts


def _make_in_maps(inputs):
    gh, scal, consts = _host_prep(inputs)
    bandc, bandp, prba, prbb = consts
    in_maps = []
    for cidx in range(NCORES):
        sl = slice(cidx * BPC, (cidx + 1) * BPC)
        in_maps.append({
            "gh": np.ascontiguousarray(gh[sl]),
            "scal": np.ascontiguousarray(scal[sl]),
            "bandc": bandc, "bandp": bandp,
            "prba": prba, "prbb": prbb,
        })
    return in_maps


def _build_nc():
    import concourse.bass as bass
    import concourse.bacc as bacc
    import concourse.mybir as mybir
    import concourse.tile as tile

    f32 = mybir.dt.float32
    f16 = mybir.dt.float16
    Copy = mybir.ActivationFunctionType.Copy
    mult = mybir.AluOpType.mult
    sub = mybir.AluOpType.subtract
    add = mybir.AluOpType.add

    nc = bacc.Bacc("TRN2", target_bir_lowering=False, debug=False,
                   num_devices=NCORES)

    GHD = nc.dram_tensor("gh", [BPC, L, 256], f16, kind="ExternalInput").ap()
    SCD = nc.dram_tensor("scal", [BPC, 128, NT * 5], f32,
                         kind="ExternalInput").ap()
    BCD = nc.dram_tensor("bandc", [128, 128], f16, kind="ExternalInput").ap()
    BPD = nc.dram_tensor("bandp", [128, 128], f16, kind="ExternalInput").ap()
    PAD = nc.dram_tensor("prba", [128, 128], f16, kind="ExternalInput").ap()
    PBD = nc.dram_tensor("prbb", [128, 512], f16, kind="ExternalInput").ap()
    OUTD = nc.dram_tensor("out", [BPC, RP, O], f16, kind="ExternalOutput").ap()

    with tile.TileContext(nc) as tc:
        with (
            tc.tile_pool(name="const", bufs=1) as cpool,
            tc.tile_pool(name="ghp", bufs=16) as ghpool,
            tc.tile_pool(name="scp", bufs=2) as scpool,
            tc.tile_pool(name="sbw", bufs=3) as sbwpool,
            tc.tile_pool(name="pp", bufs=3) as pppool,
            tc.tile_pool(name="rr", bufs=3) as rrpool,
            tc.tile_pool(name="ob", bufs=3) as obpool,
            tc.tile_pool(name="psw", bufs=2, space=bass.MemorySpace.PSUM) as pwpool,
            tc.tile_pool(name="psp", bufs=2, space=bass.MemorySpace.PSUM) as papool,
            tc.tile_pool(name="pssp", bufs=2, space=bass.MemorySpace.PSUM) as pbpool,
        ):
            bandc_t = cpool.tile([128, 128], f16, tag="bandc")
            bandp_t = cpool.tile([128, 128], f16, tag="bandp")
            prba_t = cpool.tile([128, 128], f16, tag="prba")
            prbb_t = cpool.tile([128, 512], f16, tag="prbb")

            ghcs = {}      # (b, chunk) -> tile
            scals = {}     # b -> tile

            def load_gh_chunk(b, j):
                ghc = ghpool.tile([128, 2, 256], f16, name="ghc", tag="ghc")
                nc.sync.dma_start(
                    ghc[:],
                    GHD[b].rearrange("(n p) c -> p n c", p=128)[:, 2 * j:2 * j + 2, :])
                ghcs[(b, j)] = ghc

            def load_scal(b):
                sct = scpool.tile([128, NT * 5], f32, name="sct", tag="sct")
                nc.scalar.dma_start(sct[:], SCD[b])
                scals[b] = sct

            # first chunk of batch 0 gates everything - issue it first
            load_gh_chunk(0, 0)
            load_scal(0)
            nc.scalar.dma_start(bandc_t[:], BCD)
            nc.scalar.dma_start(bandp_t[:], BPD)
            nc.scalar.dma_start(prba_t[:], PAD)
            nc.scalar.dma_start(prbb_t[:], PBD)
            for j in range(1, 8):
                load_gh_chunk(0, j)

            for b in range(BPC):
                scal = scals.pop(b)

                psw_next = None
                biggs = {}
                pps = {}
                pssps = {}

                def stage_a(n, psw_next_in):
                    gh = ghcs[(b, n // 2)][:, n % 2, :]
                    if n == 0:
                        psw = pwpool.tile([128, 128], f32, tag="psw")
                        nc.tensor.matmul(psw[:], gh[:, 0:128], bandc_t[:],
                                         start=True, stop=True)
                    else:
                        psw = psw_next_in
                        nc.tensor.matmul(psw[:], gh[:, 0:128], bandc_t[:],
                                         start=False, stop=True)
                    nxt = None
                    if n < NT - 1:
                        nxt = pwpool.tile([128, 128], f32, tag="psw")
                        nc.tensor.matmul(nxt[:], gh[:, 128:256], bandp_t[:],
                                         start=True, stop=False)
                    sbw = sbwpool.tile([128, 128], f16, tag="sbw")
                    nc.scalar.copy(sbw[:], psw[:])
                    psp = papool.tile([128, 128], f32, tag="psp")
                    nc.tensor.matmul(psp[:], sbw[:], prba_t[:],
                                     start=True, stop=True)
                    pssp = pbpool.tile([128, 576], f32, tag="pssp")
                    nc.tensor.matmul(pssp[:, 0:512], sbw[:], prbb_t[:],
                                     start=True, stop=True)
                    nc.tensor.matmul(pssp[:, 512:576], sbw[:],
                                     prba_t[:, 0:64], start=True, stop=True)
                    pp = pppool.tile([128, 128], f32, tag="pp")
                    nc.scalar.activation(pp[:], psp[:], Copy,
                                         scale=scal[:, 5 * n:5 * n + 1])
                    pps[n] = pp
                    pssps[n] = pssp
                    return nxt

                def stage_b(m):
                    pp = pps.pop(m)
                    pssp = pssps.pop(m)
                    if m % 4 == 0:
                        biggs[m // 4] = obpool.tile([128, 4 * 640], f16,
                                                    name="bigg", tag="bigg")
                    bigg = biggs[m // 4]
                    sub_t = bigg[:, (m % 4) * 640:(m % 4 + 1) * 640]
                    simm = sub_t[:, 0:64]
                    nc.vector.scalar_tensor_tensor(
                        simm, pp[:, 0:64], scal[:, 5 * m + 2:5 * m + 3],
                        pp[:, 64:128], op0=mult, op1=sub)
                    nc.vector.scalar_tensor_tensor(
                        sub_t[:, 64:640].rearrange("p (s o) -> p s o", o=64),
                        pssp[:].rearrange("p (s o) -> p s o", o=64),
                        scal[:, 5 * m + 4:5 * m + 5],
                        simm.unsqueeze(1).broadcast_to([128, 9, 64]),
                        op0=mult, op1=add)
                    nc.scalar.activation(sub_t[:, 576:640], sub_t[:, 576:640],
                                         Copy,
                                         scale=scal[:, 5 * m + 1:5 * m + 2])
                    if m % 4 == 3:
                        n0 = m - 3
                        dst = bass.AP(
                            OUTD.tensor,
                            b * RP * 64 + (9 * n0 * 128 + 1) * 64,
                            [[576, 128], [9 * 128 * 64, 4], [1, 576]])
                        src = biggs.pop(m // 4)[:].rearrange(
                            "p (j c) -> p j c", c=640)[:, :, 64:640]
                        nc.sync.dma_start(dst, src)

                for n in range(NT):
                    psw_next = stage_a(n, psw_next)
                    if b + 1 < BPC:
                        if n == 1:
                            load_scal(b + 1)
                        if n % 2 == 0:
                            load_gh_chunk(b + 1, n // 2)
                    if n >= 1:
                        stage_b(n - 1)
                    if n % 2 == 1:
                        ghcs.pop((b, n // 2))
                stage_b(NT - 1)
    nc.compile()
    return nc


_NC_CACHE = None


def kernel(**inputs):
    global _NC_CACHE
    from concourse.bass_utils import run_bass_kernel_spmd

    in_maps = _make_in_maps(inputs)
    if _NC_CACHE is None:
        _NC_CACHE = _build_nc()
    nc = _NC_CACHE

    res = run_bass_kernel_spmd(nc, in_maps, core_ids=list(range(NCORES)))
    out = np.concatenate([r["out"] for r in res.results], 0)
    out = out[:, :ROWS, :].astype(np.float32)
    out[:, 0, :] = 0.0
    return out


# revision 12
# speedup vs baseline: 2.8195x; 1.0026x over previous
"""Trainium2 Bass kernel for ContinuousConv1DSim (gnn_message_passing).

Reformulation (numpy-validated, rel err ~2.4e-3 in fp16):
  Times are re-centered per 128-event tile (C_n = t[128n+64]) so the
  cancellation t_l*A - D happens on small-magnitude quantities and the whole
  matmul chain runs in fp16:
    G_j   = f_j * npm_j              Hv1_j = (t_j - C_{n(j)})   * G_j
    Hv2_j = (t_j - C_{n(j)+1}) * G_j                      (all host, fp16)
  MM1 (PE, fp16, transposed out), per 128-l tile, psw = [A | D] channels:
    main: psw      += [G|Hv1]^T @ bandc      (one 128-wide matmul)
    halo: psw_next  = [G|Hv2]^T @ bandp      (one matmul, 2-chunk lhsT AP)
  MM2 (PE, fp16): psp = sbw^T @ prba = [A@Wt | D@Wt - A@bias]
                  pssp = sbw^T @ prbb = [u_s * A@Wt]_s
  pp   = npm * psp                  (ACT scale-copy, fp32 - keeps cancellation)
  simm = ppA*(t_l-C) - ppD          (DVE STT)
  obsim= pssp*udt + simm            (DVE STT, bcast over s)
  rr   = ppA*(t_{l+1}-C) - ppD      (DVE STT);  real = rr * npm_{l+1} (ACT)
  Per-tile block [simm | obsim(512) | real(64)]; cols 64:640 are DMA'd as 576
  contiguous fp16 per l into the output row layout (sim rows of l then real
  row of l+1), batched 4 tiles per DMA.  Output DRAM is fp16 (row-padded);
  host casts to fp32 and zeroes row 0.
  The loop is software-pipelined one tile deep: PE/ACT producer work for tile
  n issues before DVE/store work for tile n-1, so engines overlap.

Pure data parallel: batch 32 -> 8 cores x 4. All params replicated.
"""

import numpy as np

B, L, C, O, S = 32, 2048, 64, 64, 8
NCORES = 8
BPC = B // NCORES          # 4 batches per core
NT = L // 128              # 16 l-tiles per batch
ROWS = (L - 1) * (S + 1) + 1   # 18424
RP = 9 * L + 8                 # padded rows in DRAM (last lane ends at 9*(L-1)+9)


def _consts(W, bias, u):
    n = np.arange(128)
    bandc = ((n[:, None] >= n[None, :] - 7) & (n[:, None] <= n[None, :]))
    bandp = (n[:, None] >= n[None, :] + 121)
    Wt = W.T.astype(np.float16)           # (C, O)
    prba = np.zeros((128, 128), np.float16)
    prba[0:64, 0:64] = Wt                 # A@Wt
    prba[0:64, 64:128] = -bias.astype(np.float16)   # -A@bias into D''
    prba[64:128, 64:128] = Wt             # D@Wt
    prbb = np.zeros((128, 512), np.float16)
    for s in range(S):
        prbb[0:64, s * 64:(s + 1) * 64] = (u[s] * W.T).astype(np.float16)
    return (bandc.astype(np.float16), bandp.astype(np.float16), prba, prbb)


def _host_prep(inputs):
    times = np.ascontiguousarray(inputs["times"], np.float32)
    feats = np.ascontiguousarray(inputs["features"], np.float32)
    npm = inputs["non_pad_mask"].astype(np.float32)
    u = np.asarray(inputs["uniform_sample"], np.float32)
    W = np.ascontiguousarray(inputs["W"], np.float32)
    bias = np.ascontiguousarray(inputs["bias_param"], np.float32)

    consts = _consts(W, bias, u)

    Cn = times[:, 64::128]                               # (B, NT) tile centers
    C_of = np.repeat(Cn, 128, axis=1)                    # C_{n(l)}
    C_next = np.concatenate([Cn[:, 1:], Cn[:, -1:] + 1.0], axis=1)
    C_next_of = np.repeat(C_next, 128, axis=1)
    tnext = np.concatenate([times[:, 1:], np.zeros((B, 1), np.float32)], 1)
    npmn = np.concatenate([npm[:, 1:], np.zeros((B, 1), np.float32)], 1)
    udt = (tnext - times) * npm * npmn
    ttc = times - C_of
    ttcR = tnext - C_of

    G = (feats * npm[:, :, None]).astype(np.float16)
    gh = np.empty((B, L, 256), np.float16)
    gh[:, :, 0:64] = G
    gh[:, :, 64:128] = G * ttc[:, :, None].astype(np.float16)        # Hv1
    gh[:, :, 128:192] = G
    gh[:, :, 192:256] = G * (times - C_next_of)[:, :, None].astype(np.float16)

    # per-lane scalars, partition-major: scal[b, p, n*5+k]
    # k: 0=npt 1=nsh 2=ttc 3=ttcR 4=udt
    sc = np.stack([npm, npmn, ttc, ttcR, udt], axis=2)   # (B, L, 5)
    scal = np.ascontiguousarray(
        sc.reshape(B, NT, 128, 5).transpose(0, 2, 1, 3).reshape(B, 128, NT * 5)
    ).astype(np.float32)
    return gh, scal, consts


def _make_in_maps(inputs):
    gh, scal, consts = _host_prep(inputs)
    bandc, bandp, prba, prbb = consts
    in_maps = []
    for cidx in range(NCORES):
        sl = slice(cidx * BPC, (cidx + 1) * BPC)
        in_maps.append({
            "gh": np.ascontiguousarray(gh[sl]),
            "scal": np.ascontiguousarray(scal[sl]),
            "bandc": bandc, "bandp": bandp,
            "prba": prba, "prbb": prbb,
        })
    return in_maps


def _build_nc():
    import concourse.bass as bass
    import concourse.bacc as bacc
    import concourse.mybir as mybir
    import concourse.tile as tile

    f32 = mybir.dt.float32
    f16 = mybir.dt.float16
    Copy = mybir.ActivationFunctionType.Copy
    mult = mybir.AluOpType.mult
    sub = mybir.AluOpType.subtract
    add = mybir.AluOpType.add

    nc = bacc.Bacc("TRN2", target_bir_lowering=False, debug=False,
                   num_devices=NCORES)

    GHD = nc.dram_tensor("gh", [BPC, L, 256], f16, kind="ExternalInput").ap()
    SCD = nc.dram_tensor("scal", [BPC, 128, NT * 5], f32,
                         kind="ExternalInput").ap()
    BCD = nc.dram_tensor("bandc", [128, 128], f16, kind="ExternalInput").ap()
    BPD = nc.dram_tensor("bandp", [128, 128], f16, kind="ExternalInput").ap()
    PAD = nc.dram_tensor("prba", [128, 128], f16, kind="ExternalInput").ap()
    PBD = nc.dram_tensor("prbb", [128, 512], f16, kind="ExternalInput").ap()
    OUTD = nc.dram_tensor("out", [BPC, RP, O], f16, kind="ExternalOutput").ap()

    with tile.TileContext(nc) as tc:
        with (
            tc.tile_pool(name="const", bufs=1) as cpool,
            tc.tile_pool(name="ghp", bufs=16) as ghpool,
            tc.tile_pool(name="scp", bufs=2) as scpool,
            tc.tile_pool(name="sbw", bufs=3) as sbwpool,
            tc.tile_pool(name="pp", bufs=3) as pppool,
            tc.tile_pool(name="rr", bufs=3) as rrpool,
            tc.tile_pool(name="ob", bufs=3) as obpool,
            tc.tile_pool(name="psw", bufs=2, space=bass.MemorySpace.PSUM) as pwpool,
            tc.tile_pool(name="psp", bufs=2, space=bass.MemorySpace.PSUM) as papool,
            tc.tile_pool(name="pssp", bufs=2, space=bass.MemorySpace.PSUM) as pbpool,
        ):
            bandc_t = cpool.tile([128, 128], f16, tag="bandc")
            bandp_t = cpool.tile([128, 128], f16, tag="bandp")
            prba_t = cpool.tile([128, 128], f16, tag="prba")
            prbb_t = cpool.tile([128, 512], f16, tag="prbb")

            ghcs = {}      # (b, chunk) -> tile
            scals = {}     # b -> tile

            def load_gh_chunk(b, j):
                ghc = ghpool.tile([128, 2, 256], f16, name="ghc", tag="ghc")
                nc.sync.dma_start(
                    ghc[:],
                    GHD[b].rearrange("(n p) c -> p n c", p=128)[:, 2 * j:2 * j + 2, :])
                ghcs[(b, j)] = ghc

            def load_scal(b):
                sct = scpool.tile([128, NT * 5], f32, name="sct", tag="sct")
                nc.scalar.dma_start(sct[:], SCD[b])
                scals[b] = sct

            # first chunk of batch 0 gates everything - issue it first
            load_gh_chunk(0, 0)
            load_scal(0)
            nc.scalar.dma_start(bandc_t[:], BCD)
            nc.scalar.dma_start(bandp_t[:], BPD)
            nc.scalar.dma_start(prba_t[:], PAD)
            nc.scalar.dma_start(prbb_t[:], PBD)
            for j in range(1, 8):
                load_gh_chunk(0, j)

            for b in range(BPC):
                scal = scals.pop(b)

                psw_next = None
                biggs = {}
                pps = {}
                pssps = {}

                def stage_a(n, psw_next_in):
                    gh = ghcs[(b, n // 2)][:, n % 2, :]
                    if n == 0:
                        psw = pwpool.tile([128, 128], f32, tag="psw")
                        nc.tensor.matmul(psw[:], gh[:, 0:128], bandc_t[:],
                                         start=True, stop=True)
                    else:
                        psw = psw_next_in
                        nc.tensor.matmul(psw[:], gh[:, 0:128], bandc_t[:],
                                         start=False, stop=True)
                    nxt = None
                    if n < NT - 1:
                        nxt = pwpool.tile([128, 128], f32, tag="psw")
                        nc.tensor.matmul(nxt[:], gh[:, 128:256], bandp_t[:],
                                         start=True, stop=False)
                    sbw = sbwpool.tile([128, 128], f16, tag="sbw")
                    nc.scalar.copy(sbw[:], psw[:])
                    psp = papool.tile([128, 128], f32, tag="psp")
                    nc.tensor.matmul(psp[:], sbw[:], prba_t[:],
                                     start=True, stop=True)
                    pssp = pbpool.tile([128, 576], f32, tag="pssp")
                    nc.tensor.matmul(pssp[:, 0:512], sbw[:], prbb_t[:],
                                     start=True, stop=True)
                    nc.tensor.matmul(pssp[:, 512:576], sbw[:],
                                     prba_t[:, 0:64], start=True, stop=True)
                    pp = pppool.tile([128, 128], f32, tag="pp")
                    nc.scalar.activation(pp[:], psp[:], Copy,
                                         scale=scal[:, 5 * n:5 * n + 1])
                    pps[n] = pp
                    pssps[n] = pssp
                    return nxt

                def stage_b(m):
                    pp = pps.pop(m)
                    pssp = pssps.pop(m)
                    if m % 4 == 0:
                        biggs[m // 4] = obpool.tile([128, 4 * 640], f16,
                                                    name="bigg", tag="bigg")
                    bigg = biggs[m // 4]
                    sub_t = bigg[:, (m % 4) * 640:(m % 4 + 1) * 640]
                    simm = sub_t[:, 0:64]
                    nc.vector.scalar_tensor_tensor(
                        simm, pp[:, 0:64], scal[:, 5 * m + 2:5 * m + 3],
                        pp[:, 64:128], op0=mult, op1=sub)
                    nc.vector.scalar_tensor_tensor(
                        sub_t[:, 64:640].rearrange("p (s o) -> p s o", o=64),
                        pssp[:].rearrange("p (s o) -> p s o", o=64),
                        scal[:, 5 * m + 4:5 * m + 5],
                        simm.unsqueeze(1).broadcast_to([128, 9, 64]),
                        op0=mult, op1=add)
                    nc.scalar.activation(sub_t[:, 576:640], sub_t[:, 576:640],
                                         Copy,
                                         scale=scal[:, 5 * m + 1:5 * m + 2])
                    lastgrp = (b == BPC - 1 and m // 4 == 3)

                    def store(n0, ntile, coff):
                        dst = bass.AP(
                            OUTD.tensor,
                            b * RP * 64 + (9 * n0 * 128 + 1) * 64,
                            [[576, 128], [9 * 128 * 64, ntile], [1, 576]])
                        src = biggs[m // 4][:].rearrange(
                            "p (j c) -> p j c",
                            c=640)[:, coff:coff + ntile, 64:640]
                        nc.sync.dma_start(dst, src)

                    if lastgrp and m % 4 == 1:
                        store(m - 1, 2, 0)      # early first half
                    if m % 4 == 3:
                        if lastgrp:
                            store(m - 1, 2, 2)  # second half
                        else:
                            store(m - 3, 4, 0)
                        biggs.pop(m // 4)

                for n in range(NT):
                    psw_next = stage_a(n, psw_next)
                    if b + 1 < BPC:
                        if n == 1:
                            load_scal(b + 1)
                        if n % 2 == 0:
                            load_gh_chunk(b + 1, n // 2)
                    if n >= 1:
                        stage_b(n - 1)
                    if n % 2 == 1:
                        ghcs.pop((b, n // 2))
                stage_b(NT - 1)
    nc.compile()
    return nc


_NC_CACHE = None


def kernel(**inputs):
    global _NC_CACHE
    from concourse.bass_utils import run_bass_kernel_spmd

    in_maps = _make_in_maps(inputs)
    if _NC_CACHE is None:
        _NC_CACHE = _build_nc()
    nc = _NC_CACHE

    res = run_bass_kernel_spmd(nc, in_maps, core_ids=list(range(NCORES)))
    out = np.concatenate([r["out"] for r in res.results], 0)
    out = out[:, :ROWS, :].astype(np.float32)
    out[:, 0, :] = 0.0
    return out
